# revision 8
# baseline (speedup 1.0000x reference)
"""GraphTransformer2 Bass/Tile kernel for 8 Trainium2 NeuronCores.

Sequence-parallel masked attention: each core owns a 512-row slice of the
4096-token vgraph. Per block: local Q/K/V projections, AllGather of K^T/V
(bf16), attention over 32 key chunks with a multiplicative {0,1} mask applied
after exp (softmax denominators via ones-matmuls on the tensor engine,
normalization folded into the O evacuation), then Wo/LN/FFN/LN/LN all in
"T-layout" (features on partitions, tokens on the free dim).
"""

import numpy as np

import concourse.bacc as bacc
import concourse.mybir as mybir
import concourse.tile as tile
from concourse.bass_utils import run_bass_kernel_spmd

FP32 = mybir.dt.float32
BF16 = mybir.dt.bfloat16
I32 = mybir.dt.int32
AF = mybir.ActivationFunctionType
ALU = mybir.AluOpType

NCORES = 8
SZ = 4096          # n_v + 1 + n_e
D = 256
H = 4
DH = 64
NB = 3             # transformer blocks
L = SZ // NCORES   # 512 local tokens per core
KC = SZ // 128     # 32 key chunks
N_V = 2047

_CACHE = {}


def _build():
    nc = bacc.Bacc("TRN2", target_bir_lowering=False, debug=False, num_devices=NCORES)

    # ---- DRAM I/O ----
    xT_d = nc.dram_tensor("xT", [2, 128, L], FP32, kind="ExternalInput").ap()
    maskT_d = nc.dram_tensor("maskT", [KC, 128, L], I32, kind="ExternalInput").ap()
    wq_d = nc.dram_tensor("Wq", [NB, 2, 128, D], FP32, kind="ExternalInput").ap()
    wk_d = nc.dram_tensor("Wk", [NB, 2, 128, D], FP32, kind="ExternalInput").ap()
    wv_d = nc.dram_tensor("Wv", [NB, 2, 128, D], FP32, kind="ExternalInput").ap()
    wo_d = nc.dram_tensor("Wo", [NB, 2, 128, D], FP32, kind="ExternalInput").ap()
    w1_d = nc.dram_tensor("W1", [NB, 2, 128, 4 * D], FP32, kind="ExternalInput").ap()
    w2_d = nc.dram_tensor("W2", [NB, 8, 128, D], FP32, kind="ExternalInput").ap()
    bq_d = nc.dram_tensor("bq", [NB, 2, 128, 1], FP32, kind="ExternalInput").ap()
    bk_d = nc.dram_tensor("bk", [NB, 2, 128, 1], FP32, kind="ExternalInput").ap()
    bv_d = nc.dram_tensor("bv", [NB, 2, 128, 1], FP32, kind="ExternalInput").ap()
    bo_d = nc.dram_tensor("bo", [NB, 2, 128, 1], FP32, kind="ExternalInput").ap()
    b1_d = nc.dram_tensor("b1", [NB, 8, 128, 1], FP32, kind="ExternalInput").ap()
    b2_d = nc.dram_tensor("b2", [NB, 2, 128, 1], FP32, kind="ExternalInput").ap()
    lng_d = nc.dram_tensor("lng", [NB, 2, 128, 1], FP32, kind="ExternalInput").ap()
    lnb_d = nc.dram_tensor("lnb", [NB, 2, 128, 1], FP32, kind="ExternalInput").ap()
    out_d = nc.dram_tensor("out", [2, 128, L], FP32, kind="ExternalOutput").ap()

    with tile.TileContext(nc) as tc:
        _body(nc, tc, xT_d, maskT_d,
              (wq_d, wk_d, wv_d, wo_d, w1_d, w2_d),
              (bq_d, bk_d, bv_d, bo_d, b1_d, b2_d, lng_d, lnb_d),
              out_d)
    nc.compile()
    return nc


def _body(nc, tc, xT_d, maskT_d, weights_d, biases_d, out_d):
    wq_d, wk_d, wv_d, wo_d, w1_d, w2_d = weights_d
    bq_d, bk_d, bv_d, bo_d, b1_d, b2_d, lng_d, lnb_d = biases_d

    const = tc.alloc_tile_pool(name="const", bufs=1)
    ones64 = const.tile([128, 64], BF16, tag="ones64")
    nc.vector.memset(ones64[:], 1.0)
    lnones = const.tile([128, 128], FP32, tag="lnones")
    nc.vector.memset(lnones[:], 1.0 / 256.0)
    for cv, ctag in [(0.0, "zeroc"), (1e-5, "epsc")]:
        ct = const.tile([128, 1], FP32, tag=ctag)
        nc.vector.memset(ct[:], cv)
        nc.const_aps.aps[(FP32, cv)] = ct[:]

    # small per-partition bias vectors, all blocks upfront
    def bias_tiles(src, n):
        ts = []
        for b in range(NB):
            row = []
            for m in range(n):
                t = const.tile([128, 1], FP32, tag=f"bias_{id(src)}_{b}_{m}")
                nc.sync.dma_start(t[:], src[b, m])
                row.append(t)
            ts.append(row)
        return ts

    bq_s = bias_tiles(bq_d, 2)
    bk_s = bias_tiles(bk_d, 2)
    bv_s = bias_tiles(bv_d, 2)
    bo_s = bias_tiles(bo_d, 2)
    b1_s = bias_tiles(b1_d, 8)
    b2_s = bias_tiles(b2_d, 2)
    lng_s = bias_tiles(lng_d, 2)
    lnb_s = bias_tiles(lnb_d, 2)

    # ---- mask: DMA int32 slices, cast to bf16 (multiplicative {0,1}) ----
    maskbf = []
    mpool = tc.alloc_tile_pool(name="maskbf", bufs=KC)
    with tc.tile_pool(name="mstage", bufs=4) as mstage:
        for kc in range(KC):
            st = mstage.tile([128, L], I32, tag="mstage")
            nc.sync.dma_start(st[:], maskT_d[kc])
            mb = mpool.tile([128, L], BF16, tag="maskbf")
            nc.vector.tensor_copy(mb[:], st[:])
            maskbf.append(mb)

    # ---- weights: DMA f32, cast to bf16, all blocks ----
    wq_s, wk_s, wv_s, wo_s, w1_s, w2_s = [], [], [], [], [], []
    with tc.tile_pool(name="wstage", bufs=2) as wstage:
        def stage_w(dst_list, src, n, width, tag):
            for b in range(NB):
                row = []
                for m in range(n):
                    st = wstage.tile([128, width], FP32, tag="wstage")
                    nc.sync.dma_start(st[:], src[b, m])
                    wt = const.tile([128, width], BF16, tag=f"{tag}_{b}_{m}")
                    nc.vector.tensor_copy(wt[:], st[:])
                    row.append(wt)
                dst_list.append(row)
        stage_w(wq_s, wq_d, 2, D, "wq")
        stage_w(wk_s, wk_d, 2, D, "wk")
        stage_w(wv_s, wv_d, 2, D, "wv")
        stage_w(wo_s, wo_d, 2, D, "wo")
        stage_w(w1_s, w1_d, 2, 4 * D, "w1")
        stage_w(w2_s, w2_d, 8, D, "w2")

    # ---- initial local x^T (f32 -> bf16) ----
    xbf_pool = tc.alloc_tile_pool(name="xbf", bufs=4)
    xbf = []
    with tc.tile_pool(name="xstage", bufs=2) as xstage:
        for m in range(2):
            st = xstage.tile([128, L], FP32, tag="xstage")
            nc.sync.dma_start(st[:], xT_d[m])
            xb = xbf_pool.tile([128, L], BF16, tag="xbf")
            nc.vector.tensor_copy(xb[:], st[:])
            xbf.append(xb)

    # persistent pools across blocks
    qt_pool = tc.alloc_tile_pool(name="qt", bufs=2)
    ktloc_pool = tc.alloc_tile_pool(name="ktloc", bufs=2)
    vloc_pool = tc.alloc_tile_pool(name="vloc", bufs=4)
    kt_pool = tc.alloc_tile_pool(name="ktg", bufs=16)
    vg_pool = tc.alloc_tile_pool(name="vg", bufs=KC)
    p_pool = tc.alloc_tile_pool(name="pexp", bufs=2)
    pm_pool = tc.alloc_tile_pool(name="pmask", bufs=2)
    osb_pool = tc.alloc_tile_pool(name="osb", bufs=4)
    g_pool = tc.alloc_tile_pool(name="gelu", bufs=8)
    f32_pool = tc.alloc_tile_pool(name="scratch", bufs=1)
    scr2 = tc.alloc_tile_pool(name="scratch2", bufs=2)
    dram = tc.alloc_tile_pool(name="dram", bufs=2, space="DRAM")

    def layer_norm(x2, g_ap, b_ap, out_dtype, out_pool, out_tag, psum_pool):
        """T-layout layernorm over partitions via ones-matmuls.

        x2: two [128, L] f32 SBUF tiles (feature chunks). Returns 2 tiles.
        """
        sq = []
        for m in range(2):
            s = f32_pool.tile([128, L], FP32, tag="lnsq")
            nc.vector.tensor_tensor(s[:], x2[m][:], x2[m][:], op=ALU.mult)
            sq.append(s)
        mean_ps = psum_pool.tile([128, L], FP32, tag="lnmean")
        ex2_ps = psum_pool.tile([128, L], FP32, tag="lnex2")
        for m in range(2):
            nc.tensor.matmul(mean_ps[:], lhsT=lnones[:], rhs=x2[m][:],
                             start=(m == 0), stop=(m == 1))
        for m in range(2):
            nc.tensor.matmul(ex2_ps[:], lhsT=lnones[:], rhs=sq[m][:],
                             start=(m == 0), stop=(m == 1))
        mean_sb = f32_pool.tile([128, L], FP32, tag="lnmean_sb")
        nc.vector.tensor_copy(mean_sb[:], mean_ps[:])
        musq = f32_pool.tile([128, L], FP32, tag="lnmusq")
        nc.vector.tensor_tensor(musq[:], mean_sb[:], mean_sb[:], op=ALU.mult)
        var = f32_pool.tile([128, L], FP32, tag="lnvar")
        nc.vector.tensor_tensor(var[:], ex2_ps[:], musq[:], op=ALU.subtract)
        lnv = f32_pool.tile([128, L], FP32, tag="lnlnv")
        nc.scalar.activation(lnv[:], var[:], AF.Ln, bias=1e-5)
        rstd = f32_pool.tile([128, L], FP32, tag="lnrstd")
        nc.scalar.activation(rstd[:], lnv[:], AF.Exp, scale=-0.5)
        mrs = f32_pool.tile([128, L], FP32, tag="lnmrs")
        nc.vector.tensor_tensor(mrs[:], mean_sb[:], rstd[:], op=ALU.mult)
        outs = []
        for m in range(2):
            z = f32_pool.tile([128, L], FP32, tag="lnz")
            nc.vector.tensor_tensor(z[:], x2[m][:], rstd[:], op=ALU.mult)
            if g_ap is not None:
                z2 = f32_pool.tile([128, L], FP32, tag="lnz2")
                nc.vector.tensor_tensor(z2[:], z[:], mrs[:], op=ALU.subtract)
                o = out_pool.tile([128, L], out_dtype, tag=out_tag)
                nc.scalar.activation(o[:], z2[:], AF.Identity,
                                     scale=g_ap[m][:], bias=b_ap[m][:])
            else:
                o = out_pool.tile([128, L], out_dtype, tag=out_tag)
                nc.vector.tensor_tensor(o[:], z[:], mrs[:], op=ALU.subtract)
            outs.append(o)
        return outs

    for b in range(NB):
        # ---- local projections Q^T, K^T, V ----
        qt, ktloc, vloc = [], [], []
        with tc.tile_pool(name=f"psmm{b}", bufs=2, space="PSUM") as psmm:
            for m in range(2):
                q_ps = psmm.tile([128, L], FP32, tag="proj")
                for c in range(2):
                    nc.tensor.matmul(q_ps[:], lhsT=wq_s[b][c][:, 128 * m:128 * (m + 1)],
                                     rhs=xbf[c][:], start=(c == 0), stop=(c == 1))
                qtile = qt_pool.tile([128, L], BF16, tag="qt")
                nc.vector.tensor_scalar_add(qtile[:], q_ps[:], bq_s[b][m][:])
                qt.append(qtile)
                k_ps = psmm.tile([128, L], FP32, tag="proj")
                for c in range(2):
                    nc.tensor.matmul(k_ps[:], lhsT=wk_s[b][c][:, 128 * m:128 * (m + 1)],
                                     rhs=xbf[c][:], start=(c == 0), stop=(c == 1))
                ktile = ktloc_pool.tile([128, L], BF16, tag="ktloc")
                nc.vector.tensor_scalar_add(ktile[:], k_ps[:], bk_s[b][m][:])
                ktloc.append(ktile)
            for t4 in range(4):
                v_ps = psmm.tile([128, D], FP32, tag="projv")
                for c in range(2):
                    nc.tensor.matmul(v_ps[:], lhsT=xbf[c][:, 128 * t4:128 * (t4 + 1)],
                                     rhs=wv_s[b][c][:], start=(c == 0), stop=(c == 1))
                vtile = vloc_pool.tile([128, D], BF16, tag="vloc")
                nc.vector.tensor_copy(vtile[:], v_ps[:])
                vloc.append(vtile)

        # ---- AllGather K^T and V (bf16) ----
        bounce_in = dram.tile([L, L], BF16, tag="agin")
        bounce_out = dram.tile([NCORES * L, L], BF16, tag="agout")
        for m in range(2):
            nc.sync.dma_start(bounce_in[128 * m:128 * (m + 1), :], ktloc[m][:])
        for t4 in range(4):
            r0, c0 = 256 + 128 * (t4 // 2), D * (t4 % 2)
            nc.sync.dma_start(bounce_in[r0:r0 + 128, c0:c0 + D], vloc[t4][:])
        nc.gpsimd.collective_compute(
            "AllGather", ALU.bypass,
            replica_groups=[list(range(NCORES))],
            ins=[bounce_in.opt()], outs=[bounce_out.opt()],
        )
        ktg = []   # [g][m] -> [128, 512] bf16
        for g in range(NCORES):
            row = []
            for m in range(2):
                t = kt_pool.tile([128, L], BF16, tag="ktg")
                nc.sync.dma_start(t[:], bounce_out[L * g + 128 * m:L * g + 128 * (m + 1), :])
                row.append(t)
            ktg.append(row)
        vg = []    # [kc] -> [128, 256] bf16
        for kc in range(KC):
            g, t4 = kc // 4, kc % 4
            r0, c0 = L * g + 256 + 128 * (t4 // 2), D * (t4 % 2)
            t = vg_pool.tile([128, D], BF16, tag="vg")
            nc.sync.dma_start(t[:], bounce_out[r0:r0 + 128, c0:c0 + D])
            vg.append(t)

        # ---- attention, one head-pair at a time ----
        osb = []
        with (
            tc.tile_pool(name=f"psS{b}", bufs=2, space="PSUM") as psS,
            tc.tile_pool(name=f"psO{b}", bufs=2, space="PSUM") as psO,
            tc.tile_pool(name=f"psR{b}", bufs=2, space="PSUM") as psR,
        ):
            for p in range(2):
                o_ps = psO.tile([128, L], FP32, tag="ops")
                r_ps = psR.tile([128, L], FP32, tag="rps")
                for kc in range(KC):
                    g, t4 = kc // 4, kc % 4
                    ks = ktg[g][p][:, 128 * t4:128 * (t4 + 1)]
                    s_ps = psS.tile([128, 2 * L], FP32, tag="sps")
                    nc.tensor.matmul(s_ps[:, 0:L], lhsT=ks[0:64, :],
                                     rhs=qt[p][0:64, :], start=True, stop=True)
                    nc.tensor.matmul(s_ps[:, L:2 * L], lhsT=ks[64:128, :],
                                     rhs=qt[p][64:128, :], start=True, stop=True)
                    pexp = p_pool.tile([128, 2 * L], BF16, tag="pexp")
                    nc.scalar.activation(pexp[:], s_ps[:], AF.Exp, scale=0.125)
                    pmsk = pm_pool.tile([128, 2 * L], BF16, tag="pmask")
                    nc.vector.tensor_tensor(pmsk[:, 0:L], pexp[:, 0:L],
                                            maskbf[kc][:], op=ALU.mult)
                    nc.vector.tensor_tensor(pmsk[:, L:2 * L], pexp[:, L:2 * L],
                                            maskbf[kc][:], op=ALU.mult)
                    st, sp = (kc == 0), (kc == KC - 1)
                    vt = vg[kc]
                    nc.tensor.matmul(o_ps[0:64, :], lhsT=vt[:, 128 * p:128 * p + 64],
                                     rhs=pmsk[:, 0:L], start=st, stop=sp,
                                     tile_position=(0, 0))
                    nc.tensor.matmul(o_ps[64:128, :], lhsT=vt[:, 128 * p + 64:128 * (p + 1)],
                                     rhs=pmsk[:, L:2 * L], start=st, stop=sp,
                                     tile_position=(0, 64))
                    nc.tensor.matmul(r_ps[0:64, :], lhsT=ones64[:], rhs=pmsk[:, 0:L],
                                     start=st, stop=sp, tile_position=(0, 0))
                    nc.tensor.matmul(r_ps[64:128, :], lhsT=ones64[:], rhs=pmsk[:, L:2 * L],
                                     start=st, stop=sp, tile_position=(0, 64))
                # normalize: O/R with 1/R = exp(-ln(R)); add bv (softmax-avg of
                # a constant bias is the bias itself)
                lnr = f32_pool.tile([128, L], FP32, tag="lnr")
                nc.scalar.activation(lnr[:], r_ps[:], AF.Ln)
                rinv = f32_pool.tile([128, L], FP32, tag="rinv")
                nc.scalar.activation(rinv[:], lnr[:], AF.Exp, scale=-1.0)
                onrm = f32_pool.tile([128, L], FP32, tag="onrm")
                nc.vector.tensor_tensor(onrm[:], o_ps[:], rinv[:], op=ALU.mult)
                ot = osb_pool.tile([128, L], BF16, tag="osb")
                nc.vector.tensor_scalar_add(ot[:], onrm[:], bv_s[b][p][:])
                osb.append(ot)

        with tc.tile_pool(name=f"pspost{b}", bufs=2, space="PSUM") as pspost:
            # ---- Wo projection (T-layout) + bo ----
            x1 = []
            for m in range(2):
                o2 = pspost.tile([128, L], FP32, tag="post")
                for c in range(2):
                    nc.tensor.matmul(o2[:], lhsT=wo_s[b][c][:, 128 * m:128 * (m + 1)],
                                     rhs=osb[c][:], start=(c == 0), stop=(c == 1))
                xt = scr2.tile([128, L], FP32, tag="x1")
                nc.vector.tensor_scalar_add(xt[:], o2[:], bo_s[b][m][:])
                x1.append(xt)

            # ---- t = LN1(x1) with lng/lnb ----
            t_f32 = layer_norm(x1, lng_s[b], lnb_s[b], FP32, scr2, "tf32", pspost)
            t_bf = []
            for m in range(2):
                tb = scr2.tile([128, L], BF16, tag="tbf")
                nc.vector.tensor_copy(tb[:], t_f32[m][:])
                t_bf.append(tb)

            # ---- FFN: gelu(t@W1 + b1) @ W2 + b2, residual +t ----
            gsb = []
            for m8 in range(8):
                h_ps = pspost.tile([128, L], FP32, tag="post")
                for c in range(2):
                    nc.tensor.matmul(h_ps[:], lhsT=w1_s[b][c][:, 128 * m8:128 * (m8 + 1)],
                                     rhs=t_bf[c][:], start=(c == 0), stop=(c == 1))
                gt = g_pool.tile([128, L], BF16, tag="g")
                nc.scalar.activation(gt[:], h_ps[:], AF.Gelu_apprx_tanh,
                                     bias=b1_s[b][m8][:])
                gsb.append(gt)
            r2 = []
            for m in range(2):
                t2 = scr2.tile([128, L], FP32, tag="t2")
                nc.vector.tensor_scalar_add(t2[:], t_f32[m][:], b2_s[b][m][:])
                f_ps = pspost.tile([128, L], FP32, tag="post")
                for h8 in range(8):
                    nc.tensor.matmul(f_ps[:], lhsT=w2_s[b][h8][:, 128 * m:128 * (m + 1)],
                                     rhs=gsb[h8][:], start=(h8 == 0), stop=(h8 == 7))
                rr = scr2.tile([128, L], FP32, tag="resid")
                nc.vector.tensor_tensor(rr[:], f_ps[:], t2[:], op=ALU.add)
                r2.append(rr)

            # ---- x = LN2(ffn + t); vgraph = LN3(x) ----
            y2 = layer_norm(r2, lng_s[b], lnb_s[b], FP32, scr2, "y2", pspost)
            if b < NB - 1:
                xbf = layer_norm(y2, None, None, BF16, xbf_pool, "xbf", pspost)
            else:
                vout = layer_norm(y2, None, None, FP32, scr2, "vout", pspost)
                for m in range(2):
                    nc.sync.dma_start(out_d[m], vout[m][:])

    for pool in (dram, scr2, f32_pool, g_pool, osb_pool, pm_pool, p_pool, vg_pool,
                 kt_pool, vloc_pool, ktloc_pool, qt_pool, xbf_pool, mpool, const):
        pool.release()


def _prep_inputs(inputs):
    v_in, e_in, g_in = inputs["v_in"], inputs["e_in"], inputs["g_in"]
    vgraph = np.concatenate([v_in, g_in, e_in], axis=0).astype(np.float32)
    adjT = np.ascontiguousarray(inputs["adj"].T)
    shared = {
        "Wq": np.ascontiguousarray(inputs["Wq"].reshape(NB, 2, 128, D)),
        "Wk": np.ascontiguousarray(inputs["Wk"].reshape(NB, 2, 128, D)),
        "Wv": np.ascontiguousarray(inputs["Wv"].reshape(NB, 2, 128, D)),
        "Wo": np.ascontiguousarray(inputs["Wo"].reshape(NB, 2, 128, D)),
        "W1": np.ascontiguousarray(inputs["W1"].reshape(NB, 2, 128, 4 * D)),
        "W2": np.ascontiguousarray(inputs["W2"].reshape(NB, 8, 128, D)),
        "bq": np.ascontiguousarray(inputs["bq"].reshape(NB, 2, 128, 1)),
        "bk": np.ascontiguousarray(inputs["bk"].reshape(NB, 2, 128, 1)),
        "bv": np.ascontiguousarray(inputs["bv"].reshape(NB, 2, 128, 1)),
        "bo": np.ascontiguousarray(inputs["bo"].reshape(NB, 2, 128, 1)),
        "b1": np.ascontiguousarray(inputs["b1"].reshape(NB, 8, 128, 1)),
        "b2": np.ascontiguousarray(inputs["b2"].reshape(NB, 2, 128, 1)),
        "lng": np.ascontiguousarray(inputs["lng"].reshape(NB, 2, 128, 1)),
        "lnb": np.ascontiguousarray(inputs["lnb"].reshape(NB, 2, 128, 1)),
    }
    in_maps = []
    for c in range(NCORES):
        sl = slice(L * c, L * (c + 1))
        m = dict(shared)
        m["xT"] = np.ascontiguousarray(vgraph[sl].T).reshape(2, 128, L)
        m["maskT"] = np.ascontiguousarray(adjT[:, sl]).reshape(KC, 128, L)
        in_maps.append(m)
    return in_maps


def _run(inputs, trace=False):
    if "nc" not in _CACHE:
        _CACHE["nc"] = _build()
    nc = _CACHE["nc"]
    in_maps = _prep_inputs({k: np.asarray(v) for k, v in inputs.items()})
    res = run_bass_kernel_spmd(nc, in_maps, list(range(NCORES)), trace=trace)
    vg = np.concatenate(
        [np.ascontiguousarray(res.results[c]["out"].reshape(D, L).T)
         for c in range(NCORES)], axis=0)
    out = (vg[:N_V], vg[N_V + 1:], vg[N_V])
    return out, res


def kernel(**inputs):
    out, _ = _run(inputs, trace=False)
    return out


# revision 9
# speedup vs baseline: 1.0186x; 1.0186x over previous
"""GraphTransformer2 Bass/Tile kernel for 8 Trainium2 NeuronCores.

Sequence-parallel masked attention: each core owns a 512-row slice of the
4096-token vgraph. Per block: local Q/K/V projections, AllGather of K^T/V
(bf16), attention over 32 key chunks with a multiplicative {0,1} mask applied
after exp (softmax denominators via ones-matmuls on the tensor engine,
normalization folded into the O evacuation), then Wo/LN/FFN/LN/LN all in
"T-layout" (features on partitions, tokens on the free dim).
"""

import numpy as np

import concourse.bacc as bacc
import concourse.mybir as mybir
import concourse.tile as tile
from concourse.bass_utils import run_bass_kernel_spmd

FP32 = mybir.dt.float32
BF16 = mybir.dt.bfloat16
I32 = mybir.dt.int32
AF = mybir.ActivationFunctionType
ALU = mybir.AluOpType

NCORES = 8
SZ = 4096          # n_v + 1 + n_e
D = 256
H = 4
DH = 64
NB = 3             # transformer blocks
L = SZ // NCORES   # 512 local tokens per core
KC = SZ // 128     # 32 key chunks
N_V = 2047

_CACHE = {}


def _build():
    nc = bacc.Bacc("TRN2", target_bir_lowering=False, debug=False, num_devices=NCORES)

    # ---- DRAM I/O ----
    xT_d = nc.dram_tensor("xT", [2, 128, L], FP32, kind="ExternalInput").ap()
    maskT_d = nc.dram_tensor("maskT", [KC, 128, L], I32, kind="ExternalInput").ap()
    wq_d = nc.dram_tensor("Wq", [NB, 2, 128, D], FP32, kind="ExternalInput").ap()
    wk_d = nc.dram_tensor("Wk", [NB, 2, 128, D], FP32, kind="ExternalInput").ap()
    wv_d = nc.dram_tensor("Wv", [NB, 2, 128, D], FP32, kind="ExternalInput").ap()
    wo_d = nc.dram_tensor("Wo", [NB, 2, 128, D], FP32, kind="ExternalInput").ap()
    w1_d = nc.dram_tensor("W1", [NB, 2, 128, 4 * D], FP32, kind="ExternalInput").ap()
    w2_d = nc.dram_tensor("W2", [NB, 8, 128, D], FP32, kind="ExternalInput").ap()
    bq_d = nc.dram_tensor("bq", [NB, 2, 128, 1], FP32, kind="ExternalInput").ap()
    bk_d = nc.dram_tensor("bk", [NB, 2, 128, 1], FP32, kind="ExternalInput").ap()
    bv_d = nc.dram_tensor("bv", [NB, 2, 128, 1], FP32, kind="ExternalInput").ap()
    bo_d = nc.dram_tensor("bo", [NB, 2, 128, 1], FP32, kind="ExternalInput").ap()
    b1_d = nc.dram_tensor("b1", [NB, 8, 128, 1], FP32, kind="ExternalInput").ap()
    b2_d = nc.dram_tensor("b2", [NB, 2, 128, 1], FP32, kind="ExternalInput").ap()
    lng_d = nc.dram_tensor("lng", [NB, 2, 128, 1], FP32, kind="ExternalInput").ap()
    lnb_d = nc.dram_tensor("lnb", [NB, 2, 128, 1], FP32, kind="ExternalInput").ap()
    out_d = nc.dram_tensor("out", [2, 128, L], FP32, kind="ExternalOutput").ap()

    with tile.TileContext(nc) as tc:
        _body(nc, tc, xT_d, maskT_d,
              (wq_d, wk_d, wv_d, wo_d, w1_d, w2_d),
              (bq_d, bk_d, bv_d, bo_d, b1_d, b2_d, lng_d, lnb_d),
              out_d)
    nc.compile()
    return nc


def _body(nc, tc, xT_d, maskT_d, weights_d, biases_d, out_d):
    wq_d, wk_d, wv_d, wo_d, w1_d, w2_d = weights_d
    bq_d, bk_d, bv_d, bo_d, b1_d, b2_d, lng_d, lnb_d = biases_d

    const = tc.alloc_tile_pool(name="const", bufs=1)
    ones64 = const.tile([128, 64], BF16, tag="ones64")
    nc.vector.memset(ones64[:], 1.0)
    lnones = const.tile([128, 128], FP32, tag="lnones")
    nc.vector.memset(lnones[:], 1.0 / 256.0)
    for cv, ctag in [(0.0, "zeroc"), (1e-5, "epsc")]:
        ct = const.tile([128, 1], FP32, tag=ctag)
        nc.vector.memset(ct[:], cv)
        nc.const_aps.aps[(FP32, cv)] = ct[:]

    # small per-partition bias vectors, all blocks upfront
    def bias_tiles(src, n):
        ts = []
        for b in range(NB):
            row = []
            for m in range(n):
                t = const.tile([128, 1], FP32, tag=f"bias_{id(src)}_{b}_{m}")
                nc.sync.dma_start(t[:], src[b, m])
                row.append(t)
            ts.append(row)
        return ts

    bq_s = bias_tiles(bq_d, 2)
    bk_s = bias_tiles(bk_d, 2)
    bv_s = bias_tiles(bv_d, 2)
    bo_s = bias_tiles(bo_d, 2)
    b1_s = bias_tiles(b1_d, 8)
    b2_s = bias_tiles(b2_d, 2)
    lng_s = bias_tiles(lng_d, 2)
    lnb_s = bias_tiles(lnb_d, 2)

    # ---- mask: DMA int32 slices, cast to bf16 (multiplicative {0,1}) ----
    maskbf = []
    mpool = tc.alloc_tile_pool(name="maskbf", bufs=KC)
    with tc.tile_pool(name="mstage", bufs=4) as mstage:
        for kc in range(KC):
            st = mstage.tile([128, L], I32, tag="mstage")
            nc.sync.dma_start(st[:], maskT_d[kc])
            mb = mpool.tile([128, L], BF16, tag="maskbf")
            nc.vector.tensor_copy(mb[:], st[:])
            maskbf.append(mb)

    # ---- weights: DMA f32, cast to bf16, all blocks ----
    wq_s, wk_s, wv_s, wo_s, w1_s, w2_s = [], [], [], [], [], []
    with tc.tile_pool(name="wstage", bufs=2) as wstage:
        def stage_w(dst_list, src, n, width, tag):
            for b in range(NB):
                row = []
                for m in range(n):
                    st = wstage.tile([128, width], FP32, tag="wstage")
                    nc.sync.dma_start(st[:], src[b, m])
                    wt = const.tile([128, width], BF16, tag=f"{tag}_{b}_{m}")
                    nc.vector.tensor_copy(wt[:], st[:])
                    row.append(wt)
                dst_list.append(row)
        stage_w(wq_s, wq_d, 2, D, "wq")
        stage_w(wk_s, wk_d, 2, D, "wk")
        stage_w(wv_s, wv_d, 2, D, "wv")
        stage_w(wo_s, wo_d, 2, D, "wo")
        stage_w(w1_s, w1_d, 2, 4 * D, "w1")
        stage_w(w2_s, w2_d, 8, D, "w2")

    # ---- initial local x^T (f32 -> bf16) ----
    xbf_pool = tc.alloc_tile_pool(name="xbf", bufs=4)
    xbf = []
    with tc.tile_pool(name="xstage", bufs=2) as xstage:
        for m in range(2):
            st = xstage.tile([128, L], FP32, tag="xstage")
            nc.sync.dma_start(st[:], xT_d[m])
            xb = xbf_pool.tile([128, L], BF16, tag="xbf")
            nc.vector.tensor_copy(xb[:], st[:])
            xbf.append(xb)

    # persistent pools across blocks
    qt_pool = tc.alloc_tile_pool(name="qt", bufs=2)
    ktloc_pool = tc.alloc_tile_pool(name="ktloc", bufs=2)
    vloc_pool = tc.alloc_tile_pool(name="vloc", bufs=4)
    kt_pool = tc.alloc_tile_pool(name="ktg", bufs=16)
    vg_pool = tc.alloc_tile_pool(name="vg", bufs=KC)
    p_pool = tc.alloc_tile_pool(name="pexp", bufs=3)
    pm_pool = tc.alloc_tile_pool(name="pmask", bufs=3)
    osb_pool = tc.alloc_tile_pool(name="osb", bufs=4)
    g_pool = tc.alloc_tile_pool(name="gelu", bufs=8)
    f32_pool = tc.alloc_tile_pool(name="scratch", bufs=1)
    scr2 = tc.alloc_tile_pool(name="scratch2", bufs=2)
    dram = tc.alloc_tile_pool(name="dram", bufs=2, space="DRAM")

    def layer_norm(x2, g_ap, b_ap, out_dtype, out_pool, out_tag, psum_pool):
        """T-layout layernorm over partitions via ones-matmuls.

        x2: two [128, L] f32 SBUF tiles (feature chunks). Returns 2 tiles.
        """
        sq = []
        for m in range(2):
            s = f32_pool.tile([128, L], FP32, tag="lnsq")
            nc.vector.tensor_tensor(s[:], x2[m][:], x2[m][:], op=ALU.mult)
            sq.append(s)
        mean_ps = psum_pool.tile([128, L], FP32, tag="lnmean")
        ex2_ps = psum_pool.tile([128, L], FP32, tag="lnex2")
        for m in range(2):
            nc.tensor.matmul(mean_ps[:], lhsT=lnones[:], rhs=x2[m][:],
                             start=(m == 0), stop=(m == 1))
        for m in range(2):
            nc.tensor.matmul(ex2_ps[:], lhsT=lnones[:], rhs=sq[m][:],
                             start=(m == 0), stop=(m == 1))
        mean_sb = f32_pool.tile([128, L], FP32, tag="lnmean_sb")
        nc.vector.tensor_copy(mean_sb[:], mean_ps[:])
        musq = f32_pool.tile([128, L], FP32, tag="lnmusq")
        nc.vector.tensor_tensor(musq[:], mean_sb[:], mean_sb[:], op=ALU.mult)
        var = f32_pool.tile([128, L], FP32, tag="lnvar")
        nc.vector.tensor_tensor(var[:], ex2_ps[:], musq[:], op=ALU.subtract)
        lnv = f32_pool.tile([128, L], FP32, tag="lnlnv")
        nc.scalar.activation(lnv[:], var[:], AF.Ln, bias=1e-5)
        rstd = f32_pool.tile([128, L], FP32, tag="lnrstd")
        nc.scalar.activation(rstd[:], lnv[:], AF.Exp, scale=-0.5)
        mrs = f32_pool.tile([128, L], FP32, tag="lnmrs")
        nc.vector.tensor_tensor(mrs[:], mean_sb[:], rstd[:], op=ALU.mult)
        outs = []
        for m in range(2):
            z = f32_pool.tile([128, L], FP32, tag="lnz")
            nc.vector.tensor_tensor(z[:], x2[m][:], rstd[:], op=ALU.mult)
            if g_ap is not None:
                z2 = f32_pool.tile([128, L], FP32, tag="lnz2")
                nc.vector.tensor_tensor(z2[:], z[:], mrs[:], op=ALU.subtract)
                o = out_pool.tile([128, L], out_dtype, tag=out_tag)
                nc.scalar.activation(o[:], z2[:], AF.Identity,
                                     scale=g_ap[m][:], bias=b_ap[m][:])
            else:
                o = out_pool.tile([128, L], out_dtype, tag=out_tag)
                nc.vector.tensor_tensor(o[:], z[:], mrs[:], op=ALU.subtract)
            outs.append(o)
        return outs

    for b in range(NB):
        # ---- local projections Q^T, K^T, V ----
        qt, ktloc, vloc = [], [], []
        with tc.tile_pool(name=f"psmm{b}", bufs=2, space="PSUM") as psmm:
            for m in range(2):
                q_ps = psmm.tile([128, L], FP32, tag="proj")
                for c in range(2):
                    nc.tensor.matmul(q_ps[:], lhsT=wq_s[b][c][:, 128 * m:128 * (m + 1)],
                                     rhs=xbf[c][:], start=(c == 0), stop=(c == 1))
                qtile = qt_pool.tile([128, L], BF16, tag="qt")
                nc.vector.tensor_scalar_add(qtile[:], q_ps[:], bq_s[b][m][:])
                qt.append(qtile)
                k_ps = psmm.tile([128, L], FP32, tag="proj")
                for c in range(2):
                    nc.tensor.matmul(k_ps[:], lhsT=wk_s[b][c][:, 128 * m:128 * (m + 1)],
                                     rhs=xbf[c][:], start=(c == 0), stop=(c == 1))
                ktile = ktloc_pool.tile([128, L], BF16, tag="ktloc")
                nc.vector.tensor_scalar_add(ktile[:], k_ps[:], bk_s[b][m][:])
                ktloc.append(ktile)
            for t4 in range(4):
                v_ps = psmm.tile([128, D], FP32, tag="projv")
                for c in range(2):
                    nc.tensor.matmul(v_ps[:], lhsT=xbf[c][:, 128 * t4:128 * (t4 + 1)],
                                     rhs=wv_s[b][c][:], start=(c == 0), stop=(c == 1))
                vtile = vloc_pool.tile([128, D], BF16, tag="vloc")
                nc.vector.tensor_copy(vtile[:], v_ps[:])
                vloc.append(vtile)

        # ---- AllGather K^T and V (bf16) ----
        bounce_in = dram.tile([L, L], BF16, tag="agin")
        bounce_out = dram.tile([NCORES * L, L], BF16, tag="agout")
        for m in range(2):
            nc.sync.dma_start(bounce_in[128 * m:128 * (m + 1), :], ktloc[m][:])
        for t4 in range(4):
            r0, c0 = 256 + 128 * (t4 // 2), D * (t4 % 2)
            nc.sync.dma_start(bounce_in[r0:r0 + 128, c0:c0 + D], vloc[t4][:])
        nc.gpsimd.collective_compute(
            "AllGather", ALU.bypass,
            replica_groups=[list(range(NCORES))],
            ins=[bounce_in.opt()], outs=[bounce_out.opt()],
        )
        ktg = []   # [g][m] -> [128, 512] bf16
        for g in range(NCORES):
            row = []
            for m in range(2):
                t = kt_pool.tile([128, L], BF16, tag="ktg")
                nc.sync.dma_start(t[:], bounce_out[L * g + 128 * m:L * g + 128 * (m + 1), :])
                row.append(t)
            ktg.append(row)
        vg = []    # [kc] -> [128, 256] bf16
        for kc in range(KC):
            g, t4 = kc // 4, kc % 4
            r0, c0 = L * g + 256 + 128 * (t4 // 2), D * (t4 % 2)
            t = vg_pool.tile([128, D], BF16, tag="vg")
            nc.sync.dma_start(t[:], bounce_out[r0:r0 + 128, c0:c0 + D])
            vg.append(t)

        # ---- attention, one head-pair at a time ----
        osb = []
        with (
            tc.tile_pool(name=f"psS{b}", bufs=2, space="PSUM") as psS,
            tc.tile_pool(name=f"psO{b}", bufs=2, space="PSUM") as psO,
            tc.tile_pool(name=f"psR{b}", bufs=2, space="PSUM") as psR,
        ):
            for p in range(2):
                o_ps = psO.tile([128, L], FP32, tag="ops")
                r_ps = psR.tile([128, L], FP32, tag="rps")

                def qk(kc, p=p):
                    g, t4 = kc // 4, kc % 4
                    ks = ktg[g][p][:, 128 * t4:128 * (t4 + 1)]
                    s_ps = psS.tile([128, 2 * L], FP32, tag="sps")
                    nc.tensor.matmul(s_ps[:, 0:L], lhsT=ks[0:64, :],
                                     rhs=qt[p][0:64, :], start=True, stop=True)
                    nc.tensor.matmul(s_ps[:, L:2 * L], lhsT=ks[64:128, :],
                                     rhs=qt[p][64:128, :], start=True, stop=True)
                    return s_ps

                s_cur = qk(0)
                for kc in range(KC):
                    s_nxt = qk(kc + 1) if kc + 1 < KC else None
                    pexp = p_pool.tile([128, 2 * L], BF16, tag="pexp")
                    nc.scalar.activation(pexp[:], s_cur[:], AF.Exp, scale=0.125)
                    s_cur = s_nxt
                    pmsk = pm_pool.tile([128, 2 * L], BF16, tag="pmask")
                    nc.vector.tensor_tensor(pmsk[:, 0:L], pexp[:, 0:L],
                                            maskbf[kc][:], op=ALU.mult)
                    nc.vector.tensor_tensor(pmsk[:, L:2 * L], pexp[:, L:2 * L],
                                            maskbf[kc][:], op=ALU.mult)
                    st, sp = (kc == 0), (kc == KC - 1)
                    vt = vg[kc]
                    nc.tensor.matmul(o_ps[0:64, :], lhsT=vt[:, 128 * p:128 * p + 64],
                                     rhs=pmsk[:, 0:L], start=st, stop=sp,
                                     tile_position=(0, 0))
                    nc.tensor.matmul(o_ps[64:128, :], lhsT=vt[:, 128 * p + 64:128 * (p + 1)],
                                     rhs=pmsk[:, L:2 * L], start=st, stop=sp,
                                     tile_position=(0, 64))
                    nc.tensor.matmul(r_ps[0:64, :], lhsT=ones64[:], rhs=pmsk[:, 0:L],
                                     start=st, stop=sp, tile_position=(0, 0))
                    nc.tensor.matmul(r_ps[64:128, :], lhsT=ones64[:], rhs=pmsk[:, L:2 * L],
                                     start=st, stop=sp, tile_position=(0, 64))
                # normalize: O/R with 1/R = exp(-ln(R)); add bv (softmax-avg of
                # a constant bias is the bias itself)
                lnr = f32_pool.tile([128, L], FP32, tag="lnr")
                nc.scalar.activation(lnr[:], r_ps[:], AF.Ln)
                rinv = f32_pool.tile([128, L], FP32, tag="rinv")
                nc.scalar.activation(rinv[:], lnr[:], AF.Exp, scale=-1.0)
                onrm = f32_pool.tile([128, L], FP32, tag="onrm")
                nc.vector.tensor_tensor(onrm[:], o_ps[:], rinv[:], op=ALU.mult)
                ot = osb_pool.tile([128, L], BF16, tag="osb")
                nc.vector.tensor_scalar_add(ot[:], onrm[:], bv_s[b][p][:])
                osb.append(ot)

        with tc.tile_pool(name=f"pspost{b}", bufs=2, space="PSUM") as pspost:
            # ---- Wo projection (T-layout) + bo ----
            x1 = []
            for m in range(2):
                o2 = pspost.tile([128, L], FP32, tag="post")
                for c in range(2):
                    nc.tensor.matmul(o2[:], lhsT=wo_s[b][c][:, 128 * m:128 * (m + 1)],
                                     rhs=osb[c][:], start=(c == 0), stop=(c == 1))
                xt = scr2.tile([128, L], FP32, tag="x1")
                nc.vector.tensor_scalar_add(xt[:], o2[:], bo_s[b][m][:])
                x1.append(xt)

            # ---- t = LN1(x1) with lng/lnb ----
            t_f32 = layer_norm(x1, lng_s[b], lnb_s[b], FP32, scr2, "tf32", pspost)
            t_bf = []
            for m in range(2):
                tb = scr2.tile([128, L], BF16, tag="tbf")
                nc.vector.tensor_copy(tb[:], t_f32[m][:])
                t_bf.append(tb)

            # ---- FFN: gelu(t@W1 + b1) @ W2 + b2, residual +t ----
            gsb = []
            for m8 in range(8):
                h_ps = pspost.tile([128, L], FP32, tag="post")
                for c in range(2):
                    nc.tensor.matmul(h_ps[:], lhsT=w1_s[b][c][:, 128 * m8:128 * (m8 + 1)],
                                     rhs=t_bf[c][:], start=(c == 0), stop=(c == 1))
                gt = g_pool.tile([128, L], BF16, tag="g")
                nc.scalar.activation(gt[:], h_ps[:], AF.Gelu_apprx_tanh,
                                     bias=b1_s[b][m8][:])
                gsb.append(gt)
            r2 = []
            for m in range(2):
                t2 = scr2.tile([128, L], FP32, tag="t2")
                nc.vector.tensor_scalar_add(t2[:], t_f32[m][:], b2_s[b][m][:])
                f_ps = pspost.tile([128, L], FP32, tag="post")
                for h8 in range(8):
                    nc.tensor.matmul(f_ps[:], lhsT=w2_s[b][h8][:, 128 * m:128 * (m + 1)],
                                     rhs=gsb[h8][:], start=(h8 == 0), stop=(h8 == 7))
                rr = scr2.tile([128, L], FP32, tag="resid")
                nc.vector.tensor_tensor(rr[:], f_ps[:], t2[:], op=ALU.add)
                r2.append(rr)

            # ---- x = LN2(ffn + t); vgraph = LN3(x) ----
            y2 = layer_norm(r2, lng_s[b], lnb_s[b], FP32, scr2, "y2", pspost)
            if b < NB - 1:
                xbf = layer_norm(y2, None, None, BF16, xbf_pool, "xbf", pspost)
            else:
                vout = layer_norm(y2, None, None, FP32, scr2, "vout", pspost)
                for m in range(2):
                    nc.sync.dma_start(out_d[m], vout[m][:])

    for pool in (dram, scr2, f32_pool, g_pool, osb_pool, pm_pool, p_pool, vg_pool,
                 kt_pool, vloc_pool, ktloc_pool, qt_pool, xbf_pool, mpool, const):
        pool.release()


def _prep_inputs(inputs):
    v_in, e_in, g_in = inputs["v_in"], inputs["e_in"], inputs["g_in"]
    vgraph = np.concatenate([v_in, g_in, e_in], axis=0).astype(np.float32)
    adjT = np.ascontiguousarray(inputs["adj"].T)
    shared = {
        "Wq": np.ascontiguousarray(inputs["Wq"].reshape(NB, 2, 128, D)),
        "Wk": np.ascontiguousarray(inputs["Wk"].reshape(NB, 2, 128, D)),
        "Wv": np.ascontiguousarray(inputs["Wv"].reshape(NB, 2, 128, D)),
        "Wo": np.ascontiguousarray(inputs["Wo"].reshape(NB, 2, 128, D)),
        "W1": np.ascontiguousarray(inputs["W1"].reshape(NB, 2, 128, 4 * D)),
        "W2": np.ascontiguousarray(inputs["W2"].reshape(NB, 8, 128, D)),
        "bq": np.ascontiguousarray(inputs["bq"].reshape(NB, 2, 128, 1)),
        "bk": np.ascontiguousarray(inputs["bk"].reshape(NB, 2, 128, 1)),
        "bv": np.ascontiguousarray(inputs["bv"].reshape(NB, 2, 128, 1)),
        "bo": np.ascontiguousarray(inputs["bo"].reshape(NB, 2, 128, 1)),
        "b1": np.ascontiguousarray(inputs["b1"].reshape(NB, 8, 128, 1)),
        "b2": np.ascontiguousarray(inputs["b2"].reshape(NB, 2, 128, 1)),
        "lng": np.ascontiguousarray(inputs["lng"].reshape(NB, 2, 128, 1)),
        "lnb": np.ascontiguousarray(inputs["lnb"].reshape(NB, 2, 128, 1)),
    }
    in_maps = []
    for c in range(NCORES):
        sl = slice(L * c, L * (c + 1))
        m = dict(shared)
        m["xT"] = np.ascontiguousarray(vgraph[sl].T).reshape(2, 128, L)
        m["maskT"] = np.ascontiguousarray(adjT[:, sl]).reshape(KC, 128, L)
        in_maps.append(m)
    return in_maps


def _run(inputs, trace=False):
    if "nc" not in _CACHE:
        _CACHE["nc"] = _build()
    nc = _CACHE["nc"]
    in_maps = _prep_inputs({k: np.asarray(v) for k, v in inputs.items()})
    res = run_bass_kernel_spmd(nc, in_maps, list(range(NCORES)), trace=trace)
    vg = np.concatenate(
        [np.ascontiguousarray(res.results[c]["out"].reshape(D, L).T)
         for c in range(NCORES)], axis=0)
    out = (vg[:N_V], vg[N_V + 1:], vg[N_V])
    return out, res


def kernel(**inputs):
    out, _ = _run(inputs, trace=False)
    return out
